# revision 17
# baseline (speedup 1.0000x reference)
"""Adaptive linear (per-batch expert weight gather + matmul + bias) on 8 TRN2 cores.

Reference semantics:
    out[b, n, o] = sum_k x[b, n, k] * weight[indices[b], k, o] + bias[indices[b], 0, o]
with x [256, 1024, 256], indices [256], weight [1024, 256, 256], bias [1024, 1, 256].

Sharding: data-parallel over the batch dim B=256 -> 32 batches per core. The
weight/bias tables are replicated to every core; each core gathers the 32
weight tiles it needs ON DEVICE via indices-driven indirect DMA, then runs
bf16 matmuls (w stationary, x moving) accumulating in fp32 PSUM, adds the
gathered bias during the PSUM drain, and writes out.

Engine plan per core (all casts ride the SWDGE DMA datapath -- no separate
conversion passes on the compute engines):
  - gpsimd (SWDGE): weight gathers (indirect DMA, f32 table -> bf16 SBUF
           tiles, one call per batch) interleaved with x loads (f32 DRAM ->
           bf16 SBUF) at a 2-group lookahead, plus the bias indirect gather
  - tensor: bf16 matmuls, K split in two 128-partition PSUM-accumulated chunks
  - vector: PSUM drain (f=0 half) with bias add
  - scalar: PSUM drain (f=1 half) with bias add + output stores (HWDGE ring)

Layout choices (host-side, pure layout/sharding transforms):
  - x is passed per-core transposed with the contraction dim (IN) on SBUF
    partitions, interleaved even/odd: x_t[p, j, b, n] = x[b, n, 2p+j]. This
    matches the packed weight layout so no on-device transpose is needed; the
    contraction splits into two K=128 chunks (j=0: even k, j=1: odd k).
  - the weight table is passed as rows [C*128, 512]: row (c*128+p) holds
    weight[c, 2p:2p+2, :]. One indirect gather per batch (128 rows of 2KB)
    pulls w[indices[b]] into SBUF in exactly the lhsT layout.
  - output is produced as out^T ([OUT, BL*N], bf16) and upcast/transposed back
    on the host after gathering.
  - gather offset vectors (idx*128 + p) are precomputed on the host from the
    indices (pure index arithmetic; the data movement happens on device).
"""

import numpy as np

from concourse import bacc, bass, mybir, tile
from concourse.bass_utils import run_bass_kernel_spmd
from concourse.masks import make_identity

NCORES = 8
B, N, IN, OUT, C = 256, 1024, 256, 256, 1024
BL = B // NCORES          # 32 batches per core
KC = 2                    # contraction chunks (even/odd interleave planes)
MC = OUT // 128           # 2 output-partition chunks
FD = 512                  # max matmul free dim into one fp32 PSUM bank
FC = N // FD              # 2 free chunks
NB = 4                    # batches per x/out DMA group
LOOKAHEAD = 2 * NB        # gather this many batches ahead of the compute loop

COMPUTE = "bf16"          # "bf16" (fast, ~1e-3 rel err) or "f32" (exact)
OUT_BF16 = True           # write out^T as bf16 (halves store traffic)

_F32 = mybir.dt.float32
_BF16 = mybir.dt.bfloat16
_I32 = mybir.dt.int32

_nc_cache = []
_last_in_maps = None


def _build():
    nc = bacc.Bacc("TRN2", target_bir_lowering=False, debug=False, num_devices=NCORES)
    x_t = nc.dram_tensor("x_t", [128, KC * BL * N], _F32, kind="ExternalInput").ap()
    wtab = nc.dram_tensor("wtab", [C * 128, KC * OUT], _F32, kind="ExternalInput").ap()
    btab = nc.dram_tensor("btab", [C, OUT], _F32, kind="ExternalInput").ap()
    woff = nc.dram_tensor("woff", [128, BL], _I32, kind="ExternalInput").ap()
    idx = nc.dram_tensor("idx", [BL], _I32, kind="ExternalInput").ap()
    out_t = nc.dram_tensor(
        "out_t", [OUT, BL * N], _BF16 if OUT_BF16 else _F32, kind="ExternalOutput"
    ).ap()

    bf16 = COMPUTE == "bf16"
    mm_dt = _BF16 if bf16 else _F32
    o_dt = _BF16 if OUT_BF16 else _F32

    with tile.TileContext(nc) as tc:
        with (
            tc.tile_pool(name="sb", bufs=1) as sb,
            tc.tile_pool(name="wp", bufs=1) as wp,
            tc.tile_pool(name="xp", bufs=1) as xp,
            tc.tile_pool(name="op", bufs=1) as op,
            tc.tile_pool(name="psp", bufs=1, space="PSUM") as psp,
        ):
            idxt = sb.tile([BL, 1], _I32, tag="idxt", bufs=1)
            nc.sync.dma_start(idxt[:], idx[0:BL, None])
            offs = sb.tile([128, BL], _I32, tag="offs", bufs=1)
            nc.sync.dma_start(offs[:], woff[:])

            # bias: gather the 32 rows, then PE-transpose to [OUT-chunk, BL]
            ident = sb.tile([128, 128], _F32, tag="ident", bufs=1)
            make_identity(nc, ident[:])
            bsb = sb.tile([BL, OUT], _F32, tag="bsb", bufs=1)
            nc.gpsimd.indirect_dma_start(
                out=bsb[:],
                out_offset=None,
                in_=btab[:, :],
                in_offset=bass.IndirectOffsetOnAxis(ap=idxt[:, :1], axis=0),
            )
            bt = []
            for mc in range(MC):
                pst = psp.tile([128, FD], _F32, tag="mm", bufs=8, name=f"pst_{mc}")
                nc.tensor.transpose(
                    out=pst[:, :BL],
                    in_=bsb[:BL, mc * 128 : (mc + 1) * 128],
                    identity=ident[:BL, :BL],
                )
                btile = sb.tile([128, BL], _F32, tag="bt", bufs=2, name=f"bt_{mc}")
                nc.vector.tensor_copy(btile[:], pst[:, :BL])
                bt.append(btile)

            # one indirect gather per batch, casting f32 -> bf16 in the DMA
            wt = [None] * BL

            def gather(b):
                wr = wp.tile([128, KC * OUT], mm_dt, tag="wr", bufs=BL, name=f"wr_{b}")
                nc.gpsimd.indirect_dma_start(
                    out=wr[:],
                    out_offset=None,
                    in_=wtab[:, :],
                    in_offset=bass.IndirectOffsetOnAxis(ap=offs[:, b : b + 1], axis=0),
                )
                wt[b] = wr

            for b in range(BL):
                gather(b)

            for bg in range(0, BL, NB):
                xs = []
                for j in range(KC):
                    xt_ = xp.tile(
                        [128, NB * N], _F32, tag=f"x{j}", bufs=2, name=f"x_{bg}_{j}"
                    )
                    nc.sync.dma_start(
                        xt_[:], x_t[:, (j * BL + bg) * N : (j * BL + bg + NB) * N]
                    )
                    xr = xp.tile(
                        [128, NB * N], mm_dt, tag=f"xr{j}", bufs=2, name=f"xr_{bg}_{j}"
                    )
                    if j == 0:
                        nc.vector.tensor_copy(xr[:], xt_[:])
                    else:
                        nc.scalar.copy(xr[:], xt_[:])
                    xs.append(xr)
                os_ = []
                for mc in range(MC):
                    ot = op.tile(
                        [128, NB * N], o_dt, tag=f"o{mc}", bufs=2, name=f"o_{bg}_{mc}"
                    )
                    os_.append(ot)
                for j in range(NB):
                    b = bg + j
                    for mc in range(MC):
                        pss = []
                        for f in range(FC):
                            ps_mm = psp.tile(
                                [128, FD], _F32, tag="mm", bufs=8, name=f"mm_{b}_{mc}_{f}"
                            )
                            pss.append(ps_mm)
                        for kc in range(KC):
                            lhsT = wt[b][:, kc * OUT + mc * 128 : kc * OUT + (mc + 1) * 128]
                            for f in range(FC):
                                rhs = xs[kc][:, j * N + f * FD : j * N + (f + 1) * FD]
                                nc.tensor.matmul(
                                    pss[f][:],
                                    lhsT,
                                    rhs,
                                    start=(kc == 0),
                                    stop=(kc == KC - 1),
                                )
                        # drain+bias: f=0 on DVE, f=1 on ACT
                        nc.vector.tensor_tensor(
                            out=os_[mc][:, j * N + 0 * FD : j * N + 1 * FD],
                            in0=pss[0][:],
                            in1=bt[mc][:, b : b + 1].to_broadcast([128, FD]),
                            op=mybir.AluOpType.add,
                        )
                        nc.scalar.activation(
                            os_[mc][:, j * N + 1 * FD : j * N + 2 * FD],
                            pss[1][:],
                            mybir.ActivationFunctionType.Identity,
                            bias=bt[mc][:, b : b + 1],
                        )
                for mc in range(MC):
                    nc.scalar.dma_start(
                        out_t[mc * 128 : (mc + 1) * 128, bg * N : (bg + NB) * N],
                        os_[mc][:],
                    )

    nc.compile()
    return nc


def _get_nc():
    if not _nc_cache:
        _nc_cache.append(_build())
    return _nc_cache[0]


def kernel(x, indices, weight, bias):
    x = np.asarray(x, dtype=np.float32)
    idx_np = np.asarray(indices).astype(np.int64).reshape(B)
    # weight rows packed 2 IN-rows per row: row (c*128+p) = weight[c, 2p:2p+2, :]
    wtab = np.ascontiguousarray(np.asarray(weight, dtype=np.float32)).reshape(
        C * 128, KC * OUT
    )
    btab = np.ascontiguousarray(np.asarray(bias, dtype=np.float32)).reshape(C, OUT)

    nc = _get_nc()

    in_maps = []
    for c in range(NCORES):
        sl = slice(c * BL, (c + 1) * BL)
        # x_t[p, j, b, n] = x[b, n, 2p+j]
        xs = np.ascontiguousarray(
            np.transpose(x[sl].reshape(BL, N, 128, KC), (2, 3, 0, 1))
        ).reshape(128, KC * BL * N)
        il = idx_np[sl].astype(np.int32)
        woff = (
            il[None, :] * 128 + np.arange(128, dtype=np.int32)[:, None]
        ).astype(np.int32)
        in_maps.append(
            {
                "x_t": xs,
                "wtab": wtab,
                "btab": btab,
                "woff": woff,
                "idx": il,
            }
        )

    global _last_in_maps
    _last_in_maps = in_maps

    res = run_bass_kernel_spmd(nc, in_maps, core_ids=list(range(NCORES)))

    outs = []
    for c in range(NCORES):
        ot = np.asarray(res.results[c]["out_t"], dtype=np.float32).reshape(OUT, BL, N)
        outs.append(np.transpose(ot, (1, 2, 0)))
    return np.ascontiguousarray(np.concatenate(outs, axis=0))


# revision 18
# speedup vs baseline: 1.1132x; 1.1132x over previous
"""Adaptive linear (per-batch expert weight gather + matmul + bias) on 8 TRN2 cores.

Reference semantics:
    out[b, n, o] = sum_k x[b, n, k] * weight[indices[b], k, o] + bias[indices[b], 0, o]
with x [256, 1024, 256], indices [256], weight [1024, 256, 256], bias [1024, 1, 256].

Sharding: data-parallel over the batch dim B=256 -> 32 batches per core. The
weight/bias tables are replicated to every core; each core gathers the 32
weight tiles it needs ON DEVICE via indices-driven indirect DMA, then runs
bf16 matmuls (w stationary, x moving) accumulating in fp32 PSUM, adds the
gathered bias during the PSUM drain, and writes out.

Engine plan per core (all casts ride the SWDGE DMA datapath -- no separate
conversion passes on the compute engines):
  - gpsimd (SWDGE): weight gathers (indirect DMA, f32 table -> bf16 SBUF
           tiles, one call per batch) interleaved with x loads (f32 DRAM ->
           bf16 SBUF) at a 2-group lookahead, plus the bias indirect gather
  - tensor: bf16 matmuls, K split in two 128-partition PSUM-accumulated chunks
  - vector: PSUM drain (f=0 half) with bias add
  - scalar: PSUM drain (f=1 half) with bias add + output stores (HWDGE ring)

Layout choices (host-side, pure layout/sharding transforms):
  - x is passed per-core transposed with the contraction dim (IN) on SBUF
    partitions, interleaved even/odd: x_t[p, j, b, n] = x[b, n, 2p+j]. This
    matches the packed weight layout so no on-device transpose is needed; the
    contraction splits into two K=128 chunks (j=0: even k, j=1: odd k).
  - the weight table is passed as rows [C*128, 512]: row (c*128+p) holds
    weight[c, 2p:2p+2, :]. One indirect gather per batch (128 rows of 2KB)
    pulls w[indices[b]] into SBUF in exactly the lhsT layout.
  - output is produced as out^T ([OUT, BL*N], bf16) and upcast/transposed back
    on the host after gathering.
  - gather offset vectors (idx*128 + p) are precomputed on the host from the
    indices (pure index arithmetic; the data movement happens on device).
"""

import numpy as np

from concourse import bacc, bass, mybir, tile
from concourse.bass_utils import run_bass_kernel_spmd
from concourse.masks import make_identity

NCORES = 8
B, N, IN, OUT, C = 256, 1024, 256, 256, 1024
BL = B // NCORES          # 32 batches per core
KC = 2                    # contraction chunks (even/odd interleave planes)
MC = OUT // 128           # 2 output-partition chunks
FD = 512                  # max matmul free dim into one fp32 PSUM bank
FC = N // FD              # 2 free chunks
NB = 4                    # batches per x/out DMA group
LOOKAHEAD = 2 * NB        # gather this many batches ahead of the compute loop

COMPUTE = "bf16"          # "bf16" (fast, ~1e-3 rel err) or "f32" (exact)
OUT_BF16 = True           # write out^T as bf16 (halves store traffic)

_F32 = mybir.dt.float32
_BF16 = mybir.dt.bfloat16
_I32 = mybir.dt.int32

_nc_cache = []
_last_in_maps = None


def _build():
    nc = bacc.Bacc("TRN2", target_bir_lowering=False, debug=False, num_devices=NCORES)
    x_t = nc.dram_tensor("x_t", [128, KC * BL * N], _F32, kind="ExternalInput").ap()
    wtab = nc.dram_tensor("wtab", [C * 128, KC * OUT], _F32, kind="ExternalInput").ap()
    btab = nc.dram_tensor("btab", [C, OUT], _F32, kind="ExternalInput").ap()
    woff = nc.dram_tensor("woff", [128, BL], _I32, kind="ExternalInput").ap()
    idx = nc.dram_tensor("idx", [BL], _I32, kind="ExternalInput").ap()
    out_t = nc.dram_tensor(
        "out_t", [OUT, BL * N], _BF16 if OUT_BF16 else _F32, kind="ExternalOutput"
    ).ap()

    bf16 = COMPUTE == "bf16"
    mm_dt = _BF16 if bf16 else _F32
    o_dt = _BF16 if OUT_BF16 else _F32

    with tile.TileContext(nc) as tc:
        with (
            tc.tile_pool(name="sb", bufs=1) as sb,
            tc.tile_pool(name="wp", bufs=1) as wp,
            tc.tile_pool(name="xp", bufs=1) as xp,
            tc.tile_pool(name="op", bufs=1) as op,
            tc.tile_pool(name="psp", bufs=1, space="PSUM") as psp,
        ):
            idxt = sb.tile([BL, 1], _I32, tag="idxt", bufs=1)
            nc.sync.dma_start(idxt[:], idx[0:BL, None])
            offs = sb.tile([128, BL], _I32, tag="offs", bufs=1)
            nc.sync.dma_start(offs[:], woff[:])

            # bias: gather the 32 rows, then PE-transpose to [OUT-chunk, BL]
            ident = sb.tile([128, 128], _F32, tag="ident", bufs=1)
            make_identity(nc, ident[:])
            bsb = sb.tile([BL, OUT], _F32, tag="bsb", bufs=1)
            nc.gpsimd.indirect_dma_start(
                out=bsb[:],
                out_offset=None,
                in_=btab[:, :],
                in_offset=bass.IndirectOffsetOnAxis(ap=idxt[:, :1], axis=0),
            )
            bt = []
            for mc in range(MC):
                pst = psp.tile([128, FD], _F32, tag="mm", bufs=8, name=f"pst_{mc}")
                nc.tensor.transpose(
                    out=pst[:, :BL],
                    in_=bsb[:BL, mc * 128 : (mc + 1) * 128],
                    identity=ident[:BL, :BL],
                )
                btile = sb.tile([128, BL], _F32, tag="bt", bufs=2, name=f"bt_{mc}")
                nc.vector.tensor_copy(btile[:], pst[:, :BL])
                bt.append(btile)

            # one indirect gather per batch, casting f32 -> bf16 in the DMA
            wt = [None] * BL

            def gather(b):
                wr = wp.tile([128, KC * OUT], mm_dt, tag="wr", bufs=BL, name=f"wr_{b}")
                nc.gpsimd.indirect_dma_start(
                    out=wr[:],
                    out_offset=None,
                    in_=wtab[:, :],
                    in_offset=bass.IndirectOffsetOnAxis(ap=offs[:, b : b + 1], axis=0),
                )
                wt[b] = wr

            for b in range(BL):
                gather(b)

            for bg in range(0, BL, NB):
                # one f32 DMA per j-plane, then per-batch casts so each batch's
                # matmuls unblock as soon as its own slice is converted
                xs = []
                for j in range(KC):
                    xt_ = xp.tile(
                        [128, NB * N], _F32, tag=f"x{j}", bufs=2, name=f"x_{bg}_{j}"
                    )
                    nc.sync.dma_start(
                        xt_[:], x_t[:, (j * BL + bg) * N : (j * BL + bg + NB) * N]
                    )
                    per_b = []
                    for j2 in range(NB):
                        xr = xp.tile(
                            [128, N], mm_dt, tag=f"xr{j}_{j2}", bufs=2,
                            name=f"xr_{bg}_{j}_{j2}",
                        )
                        if j == 0:
                            nc.vector.tensor_copy(
                                xr[:], xt_[:, j2 * N : (j2 + 1) * N]
                            )
                        else:
                            nc.scalar.copy(xr[:], xt_[:, j2 * N : (j2 + 1) * N])
                        per_b.append(xr)
                    xs.append(per_b)
                os_ = []
                for mc in range(MC):
                    ot = op.tile(
                        [128, NB * N], o_dt, tag=f"o{mc}", bufs=2, name=f"o_{bg}_{mc}"
                    )
                    os_.append(ot)
                for j in range(NB):
                    b = bg + j
                    for mc in range(MC):
                        pss = []
                        for f in range(FC):
                            ps_mm = psp.tile(
                                [128, FD], _F32, tag="mm", bufs=8, name=f"mm_{b}_{mc}_{f}"
                            )
                            pss.append(ps_mm)
                        for kc in range(KC):
                            lhsT = wt[b][:, kc * OUT + mc * 128 : kc * OUT + (mc + 1) * 128]
                            for f in range(FC):
                                rhs = xs[kc][j][:, f * FD : (f + 1) * FD]
                                nc.tensor.matmul(
                                    pss[f][:],
                                    lhsT,
                                    rhs,
                                    start=(kc == 0),
                                    stop=(kc == KC - 1),
                                )
                        # drain+bias: f=0 on DVE, f=1 on ACT
                        nc.vector.tensor_tensor(
                            out=os_[mc][:, j * N + 0 * FD : j * N + 1 * FD],
                            in0=pss[0][:],
                            in1=bt[mc][:, b : b + 1].to_broadcast([128, FD]),
                            op=mybir.AluOpType.add,
                        )
                        nc.scalar.activation(
                            os_[mc][:, j * N + 1 * FD : j * N + 2 * FD],
                            pss[1][:],
                            mybir.ActivationFunctionType.Identity,
                            bias=bt[mc][:, b : b + 1],
                        )
                for mc in range(MC):
                    nc.scalar.dma_start(
                        out_t[mc * 128 : (mc + 1) * 128, bg * N : (bg + NB) * N],
                        os_[mc][:],
                    )

    nc.compile()
    return nc


def _get_nc():
    if not _nc_cache:
        _nc_cache.append(_build())
    return _nc_cache[0]


def kernel(x, indices, weight, bias):
    x = np.asarray(x, dtype=np.float32)
    idx_np = np.asarray(indices).astype(np.int64).reshape(B)
    # weight rows packed 2 IN-rows per row: row (c*128+p) = weight[c, 2p:2p+2, :]
    wtab = np.ascontiguousarray(np.asarray(weight, dtype=np.float32)).reshape(
        C * 128, KC * OUT
    )
    btab = np.ascontiguousarray(np.asarray(bias, dtype=np.float32)).reshape(C, OUT)

    nc = _get_nc()

    in_maps = []
    for c in range(NCORES):
        sl = slice(c * BL, (c + 1) * BL)
        # x_t[p, j, b, n] = x[b, n, 2p+j]
        xs = np.ascontiguousarray(
            np.transpose(x[sl].reshape(BL, N, 128, KC), (2, 3, 0, 1))
        ).reshape(128, KC * BL * N)
        il = idx_np[sl].astype(np.int32)
        woff = (
            il[None, :] * 128 + np.arange(128, dtype=np.int32)[:, None]
        ).astype(np.int32)
        in_maps.append(
            {
                "x_t": xs,
                "wtab": wtab,
                "btab": btab,
                "woff": woff,
                "idx": il,
            }
        )

    global _last_in_maps
    _last_in_maps = in_maps

    res = run_bass_kernel_spmd(nc, in_maps, core_ids=list(range(NCORES)))

    outs = []
    for c in range(NCORES):
        ot = np.asarray(res.results[c]["out_t"], dtype=np.float32).reshape(OUT, BL, N)
        outs.append(np.transpose(ot, (1, 2, 0)))
    return np.ascontiguousarray(np.concatenate(outs, axis=0))


# revision 19
# speedup vs baseline: 1.1921x; 1.0709x over previous
"""Adaptive linear (per-batch expert weight gather + matmul + bias) on 8 TRN2 cores.

Reference semantics:
    out[b, n, o] = sum_k x[b, n, k] * weight[indices[b], k, o] + bias[indices[b], 0, o]
with x [256, 1024, 256], indices [256], weight [1024, 256, 256], bias [1024, 1, 256].

Sharding: data-parallel over the batch dim B=256 -> 32 batches per core. The
weight/bias tables are replicated to every core; each core gathers the 32
weight tiles it needs ON DEVICE via indices-driven indirect DMA, then runs
bf16 matmuls (w stationary, x moving) accumulating in fp32 PSUM, adds the
gathered bias during the PSUM drain, and writes out.

Engine plan per core (all casts ride the SWDGE DMA datapath -- no separate
conversion passes on the compute engines):
  - gpsimd (SWDGE): weight gathers (indirect DMA, f32 table -> bf16 SBUF
           tiles, one call per batch) interleaved with x loads (f32 DRAM ->
           bf16 SBUF) at a 2-group lookahead, plus the bias indirect gather
  - tensor: bf16 matmuls, K split in two 128-partition PSUM-accumulated chunks
  - vector: PSUM drain (f=0 half) with bias add
  - scalar: PSUM drain (f=1 half) with bias add + output stores (HWDGE ring)

Layout choices (host-side, pure layout/sharding transforms):
  - x is passed per-core transposed with the contraction dim (IN) on SBUF
    partitions, interleaved even/odd: x_t[p, j, b, n] = x[b, n, 2p+j]. This
    matches the packed weight layout so no on-device transpose is needed; the
    contraction splits into two K=128 chunks (j=0: even k, j=1: odd k).
  - the weight table is passed as rows [C*128, 512]: row (c*128+p) holds
    weight[c, 2p:2p+2, :]. One indirect gather per batch (128 rows of 2KB)
    pulls w[indices[b]] into SBUF in exactly the lhsT layout.
  - output is produced as out^T ([OUT, BL*N], bf16) and upcast/transposed back
    on the host after gathering.
  - gather offset vectors (idx*128 + p) are precomputed on the host from the
    indices (pure index arithmetic; the data movement happens on device).
"""

import numpy as np

from concourse import bacc, bass, mybir, tile
from concourse.bass_utils import run_bass_kernel_spmd
from concourse.masks import make_identity

NCORES = 8
B, N, IN, OUT, C = 256, 1024, 256, 256, 1024
BL = B // NCORES          # 32 batches per core
KC = 2                    # contraction chunks (even/odd interleave planes)
MC = OUT // 128           # 2 output-partition chunks
FD = 512                  # max matmul free dim into one fp32 PSUM bank
FC = N // FD              # 2 free chunks
NB = 4                    # batches per x/out DMA group
LOOKAHEAD = 2 * NB        # gather this many batches ahead of the compute loop

COMPUTE = "bf16"          # "bf16" (fast, ~1e-3 rel err) or "f32" (exact)
OUT_BF16 = True           # write out^T as bf16 (halves store traffic)

_F32 = mybir.dt.float32
_BF16 = mybir.dt.bfloat16
_I32 = mybir.dt.int32

_nc_cache = []
_last_in_maps = None


def _build():
    nc = bacc.Bacc("TRN2", target_bir_lowering=False, debug=False, num_devices=NCORES)
    x_t = nc.dram_tensor("x_t", [128, KC * BL * N], _F32, kind="ExternalInput").ap()
    wtab = nc.dram_tensor("wtab", [C * 128, KC * OUT], _F32, kind="ExternalInput").ap()
    btab = nc.dram_tensor("btab", [C, OUT], _F32, kind="ExternalInput").ap()
    woff = nc.dram_tensor("woff", [128, BL], _I32, kind="ExternalInput").ap()
    idx = nc.dram_tensor("idx", [BL], _I32, kind="ExternalInput").ap()
    out_t = nc.dram_tensor(
        "out_t", [OUT, BL * N], _BF16 if OUT_BF16 else _F32, kind="ExternalOutput"
    ).ap()

    bf16 = COMPUTE == "bf16"
    mm_dt = _BF16 if bf16 else _F32
    o_dt = _BF16 if OUT_BF16 else _F32

    with tile.TileContext(nc) as tc:
        with (
            tc.tile_pool(name="sb", bufs=1) as sb,
            tc.tile_pool(name="wp", bufs=1) as wp,
            tc.tile_pool(name="xp", bufs=1) as xp,
            tc.tile_pool(name="op", bufs=1) as op,
            tc.tile_pool(name="psp", bufs=1, space="PSUM") as psp,
        ):
            idxt = sb.tile([BL, 1], _I32, tag="idxt", bufs=1)
            nc.sync.dma_start(idxt[:], idx[0:BL, None])
            offs = sb.tile([128, BL], _I32, tag="offs", bufs=1)
            nc.sync.dma_start(offs[:], woff[:])

            # bias: gather the 32 rows, then PE-transpose to [OUT-chunk, BL]
            ident = sb.tile([128, 128], _F32, tag="ident", bufs=1)
            make_identity(nc, ident[:])
            bsb = sb.tile([BL, OUT], _F32, tag="bsb", bufs=1)
            nc.gpsimd.indirect_dma_start(
                out=bsb[:],
                out_offset=None,
                in_=btab[:, :],
                in_offset=bass.IndirectOffsetOnAxis(ap=idxt[:, :1], axis=0),
            )
            bt = []
            for mc in range(MC):
                pst = psp.tile([128, N], _F32, tag="mm", bufs=4, name=f"pst_{mc}")
                nc.tensor.transpose(
                    out=pst[:, :BL],
                    in_=bsb[:BL, mc * 128 : (mc + 1) * 128],
                    identity=ident[:BL, :BL],
                )
                btile = sb.tile([128, BL], _F32, tag="bt", bufs=2, name=f"bt_{mc}")
                nc.vector.tensor_copy(btile[:], pst[:, :BL])
                bt.append(btile)

            # one indirect gather per batch, casting f32 -> bf16 in the DMA
            wt = [None] * BL

            def gather(b):
                wr = wp.tile([128, KC * OUT], mm_dt, tag="wr", bufs=BL, name=f"wr_{b}")
                nc.gpsimd.indirect_dma_start(
                    out=wr[:],
                    out_offset=None,
                    in_=wtab[:, :],
                    in_offset=bass.IndirectOffsetOnAxis(ap=offs[:, b : b + 1], axis=0),
                )
                wt[b] = wr

            for b in range(BL):
                gather(b)

            for bg in range(0, BL, NB):
                # one f32 DMA per j-plane, then per-batch casts so each batch's
                # matmuls unblock as soon as its own slice is converted
                xs = []
                for j in range(KC):
                    xt_ = xp.tile(
                        [128, NB * N], _F32, tag=f"x{j}", bufs=2, name=f"x_{bg}_{j}"
                    )
                    nc.sync.dma_start(
                        xt_[:], x_t[:, (j * BL + bg) * N : (j * BL + bg + NB) * N]
                    )
                    per_b = []
                    for j2 in range(NB):
                        xr = xp.tile(
                            [128, N], mm_dt, tag=f"xr{j}_{j2}", bufs=2,
                            name=f"xr_{bg}_{j}_{j2}",
                        )
                        if j == 0:
                            nc.vector.tensor_copy(
                                xr[:], xt_[:, j2 * N : (j2 + 1) * N]
                            )
                        else:
                            nc.scalar.copy(xr[:], xt_[:, j2 * N : (j2 + 1) * N])
                        per_b.append(xr)
                    xs.append(per_b)
                os_ = []
                for mc in range(MC):
                    ot = op.tile(
                        [128, NB * N], o_dt, tag=f"o{mc}", bufs=2, name=f"o_{bg}_{mc}"
                    )
                    os_.append(ot)
                for j in range(NB):
                    b = bg + j
                    for mc in range(MC):
                        # one 2-bank PSUM tile per (batch, mc); the two f-chunk
                        # matmul groups write its halves
                        ps_mm = psp.tile(
                            [128, N], _F32, tag="mm", bufs=4, name=f"mm_{b}_{mc}"
                        )
                        for kc in range(KC):
                            lhsT = wt[b][:, kc * OUT + mc * 128 : kc * OUT + (mc + 1) * 128]
                            for f in range(FC):
                                rhs = xs[kc][j][:, f * FD : (f + 1) * FD]
                                nc.tensor.matmul(
                                    ps_mm[:, f * FD : (f + 1) * FD],
                                    lhsT,
                                    rhs,
                                    start=(kc == 0),
                                    stop=(kc == KC - 1),
                                )
                        # drain+bias: mc=0 on DVE, mc=1 on ACT
                        if mc == 0:
                            nc.vector.tensor_tensor(
                                out=os_[mc][:, j * N : (j + 1) * N],
                                in0=ps_mm[:],
                                in1=bt[mc][:, b : b + 1].to_broadcast([128, N]),
                                op=mybir.AluOpType.add,
                            )
                        else:
                            nc.scalar.activation(
                                os_[mc][:, j * N : (j + 1) * N],
                                ps_mm[:],
                                mybir.ActivationFunctionType.Identity,
                                bias=bt[mc][:, b : b + 1],
                            )
                for mc in range(MC):
                    nc.scalar.dma_start(
                        out_t[mc * 128 : (mc + 1) * 128, bg * N : (bg + NB) * N],
                        os_[mc][:],
                    )

    nc.compile()
    return nc


def _get_nc():
    if not _nc_cache:
        _nc_cache.append(_build())
    return _nc_cache[0]


def kernel(x, indices, weight, bias):
    x = np.asarray(x, dtype=np.float32)
    idx_np = np.asarray(indices).astype(np.int64).reshape(B)
    # weight rows packed 2 IN-rows per row: row (c*128+p) = weight[c, 2p:2p+2, :]
    wtab = np.ascontiguousarray(np.asarray(weight, dtype=np.float32)).reshape(
        C * 128, KC * OUT
    )
    btab = np.ascontiguousarray(np.asarray(bias, dtype=np.float32)).reshape(C, OUT)

    nc = _get_nc()

    in_maps = []
    for c in range(NCORES):
        sl = slice(c * BL, (c + 1) * BL)
        # x_t[p, j, b, n] = x[b, n, 2p+j]
        xs = np.ascontiguousarray(
            np.transpose(x[sl].reshape(BL, N, 128, KC), (2, 3, 0, 1))
        ).reshape(128, KC * BL * N)
        il = idx_np[sl].astype(np.int32)
        woff = (
            il[None, :] * 128 + np.arange(128, dtype=np.int32)[:, None]
        ).astype(np.int32)
        in_maps.append(
            {
                "x_t": xs,
                "wtab": wtab,
                "btab": btab,
                "woff": woff,
                "idx": il,
            }
        )

    global _last_in_maps
    _last_in_maps = in_maps

    res = run_bass_kernel_spmd(nc, in_maps, core_ids=list(range(NCORES)))

    outs = []
    for c in range(NCORES):
        ot = np.asarray(res.results[c]["out_t"], dtype=np.float32).reshape(OUT, BL, N)
        outs.append(np.transpose(ot, (1, 2, 0)))
    return np.ascontiguousarray(np.concatenate(outs, axis=0))
